# revision 14
# baseline (speedup 1.0000x reference)
"""HMM window log-likelihood on 8 NeuronCores (data-parallel over batch).

Math: reference computes, per batch column b,
    y[b] = exp(logsumexp_i x_T[b,i]),  x via log-space forward recursion.
Equivalently in linear space with row-normalized transition matrices
W_t = exp(w[t-1]) / rowsum, emission table L = softmax(distros, axis=1):
    y[b] = 1^T diag(em_T) W_T ... diag(em_1) W_1 em_0
Evaluated as a BACKWARD recursion (avoids transposing W on device):
    beta_L = 1;  beta_{t-1} = W_t'^T (em_t . beta_t)
    y[b] = sum_i em_0[i,b] beta_0[i,b]
where W_t' carries the row-normalization AND per-step rescale factors g_t
(host-computed from column 0 in f64) folded into its rows. Per step the
device does:
    em(t)  = dlt^T G_t            (PE matmul, K=10 indicator trick)
    em_sb  = copy(em)             (ACT drain PSUM->SBUF bf16, off-chain)
    c(t)   = em_sb . beta(t)      (DVE, one PSUM operand; two 256-col chains)
    beta() = wt'[t-1]^T c(t)      (PE matmul)
em[i,b] = L[i, bin(b,t)] via dL[i,k] = L[i,k]-L[i,k-1] against cumulative
indicators G[k,b] = [bin(b,t) >= k].
Device returns colsum[b] = y[b] * prod(g); host: lnY = log(colsum)+C.
The true lnY is ~ -584.6 for these inputs, so y underflows f32 to 0.0 —
exactly matching the reference (which also underflows in f32).
"""
import sys, os
for p in ("/opt/trn_rl_repo",):
    if p not in sys.path:
        sys.path.insert(0, p)
import numpy as np
import ml_dtypes

from concourse import bass, bacc, mybir
from concourse.tile import TileContext
from concourse.bass_utils import run_bass_kernel_spmd

W, L, B, NB = 128, 256, 4096, 10
NCORES = 8
BC = B // NCORES          # 512 batch cols per core
BH = BC // 2              # two chains of 256 cols
TBLK = 16                 # G streaming block (t's per DMA)
EMLOOK = 3                # em matmul lookahead (PSUM tiles)

LAST_LNY = None           # debug: device-derived lnY per batch col
LAST_RESULTS = None       # debug: raw BassKernelResults

_CACHED = None            # (nc,) build cache


def _build_nc():
    nc = bacc.Bacc("TRN2", target_bir_lowering=False, debug=False,
                   num_devices=NCORES)
    bf16, f32 = mybir.dt.bfloat16, mybir.dt.float32
    fp8 = mybir.dt.float8e4
    DR = mybir.MatmulPerfMode.DoubleRow
    Copy = mybir.ActivationFunctionType.Copy

    wt = nc.dram_tensor("wt", [W, L - 1, W], bf16, kind="ExternalInput")
    dlt = nc.dram_tensor("dlt", [NB, 2, W], fp8, kind="ExternalInput")
    g10 = nc.dram_tensor("g10", [NB, 2, L, BC], fp8, kind="ExternalInput")
    ones = nc.dram_tensor("ones", [W, 1], bf16, kind="ExternalInput")
    colsum = nc.dram_tensor("colsum", [1, BC], f32, kind="ExternalOutput")

    with TileContext(nc) as tc:
        with tc.sbuf_pool(name="sb", bufs=2) as sb, \
                tc.psum_pool(name="ps", bufs=2) as ps:
            dlt_sb = sb.tile([NB, 2, W], fp8, bufs=1)
            nc.sync.dma_start(dlt_sb, dlt.ap())
            ones_sb = sb.tile([W, 1], bf16, bufs=1)
            nc.sync.dma_start(ones_sb, ones.ap())

            # all 255 transition matrices resident; chunked DMAs in backward
            # order so the scan can start as soon as the tail chunk lands
            wt_sb = sb.tile([W, L - 1, W], bf16, bufs=1)
            for cc in range((L - 1 + 7) // 8 - 1, -1, -1):
                t0 = cc * 8
                cnt = min(8, L - 1 - t0)
                nc.sync.dma_start(wt_sb[:, t0:t0 + cnt, :],
                                  wt.ap()[:, t0:t0 + cnt, :])

            g_tiles = {}

            def ensure_g(blk):
                if 0 <= blk < L // TBLK and blk not in g_tiles:
                    gt = sb.tile([NB, 2, TBLK, BC], fp8, tag="g", bufs=3)
                    nc.sync.dma_start(
                        gt, g10.ap()[:, :, blk * TBLK:(blk + 1) * TBLK, :])
                    g_tiles[blk] = gt

            def em_matmul(t):
                blk, ti = t // TBLK, t % TBLK
                ensure_g(blk)
                ensure_g(blk - 1)       # prefetch next block (scan backward)
                e = ps.tile([W, BC], f32, tag="em", bufs=EMLOOK)
                nc.tensor.matmul(e, dlt_sb, g_tiles[blk][:, :, ti, :],
                                 start=True, stop=True, perf_mode=DR)
                return e

            def em_drain(e):
                es = sb.tile([W, BC], bf16, tag="emsb", bufs=EMLOOK)
                nc.scalar.activation(es, e, Copy)
                return es

            em_q = [em_matmul(L - 1 - i) for i in range(EMLOOK)]
            es_q = [em_drain(em_q.pop(0)) for _ in range(2)]

            beta_ps = [None, None]
            cs_ps = None
            for t in range(L - 1, -1, -1):
                em_sb = es_q.pop(0)
                c_sb = [None, None]
                for h in (0, 1):
                    lo = h * BH
                    if t == L - 1:
                        c_sb[h] = em_sb[:, lo:lo + BH]
                    else:
                        c = sb.tile([W, BH], bf16, tag=f"c{h}", bufs=3)
                        nc.vector.tensor_mul(c, em_sb[:, lo:lo + BH],
                                             beta_ps[h])
                        c_sb[h] = c
                if t > 0:
                    for h in (0, 1):
                        b = ps.tile([W, BH], f32, tag=f"b{h}", bufs=2)
                        nc.tensor.matmul(b, wt_sb[:, t - 1, :], c_sb[h],
                                         start=True, stop=True)
                        beta_ps[h] = b
                else:
                    cs_ps = ps.tile([1, BC], f32, tag="cs", bufs=1)
                    for h in (0, 1):
                        nc.tensor.matmul(cs_ps[:, h * BH:(h + 1) * BH],
                                         ones_sb, c_sb[h],
                                         start=True, stop=True)
                # refill lookahead queues (emitted after this t's chain ops
                # so the PE/ACT queues interleave chain work with lookahead)
                if t - EMLOOK >= 0:
                    em_q.append(em_matmul(t - EMLOOK))
                if t - 2 >= 0:
                    es_q.append(em_drain(em_q.pop(0)))

            cs_sb = sb.tile([1, BC], f32, bufs=1)
            nc.vector.tensor_copy(cs_sb, cs_ps)
            nc.sync.dma_start(colsum.ap(), cs_sb)
    nc.compile()
    return nc


def _host_prep(data, input_distros, dense_layer_weights):
    f64 = np.float64
    we = np.exp(dense_layer_weights.astype(f64))           # (255,W,W)
    rowsum = we.sum(axis=2)                                # (255,W)
    recip = 1.0 / rowsum
    d = input_distros.astype(f64)
    d = d - d.max(axis=1, keepdims=True)
    e = np.exp(d)
    Ll = e / e.sum(axis=1, keepdims=True)                  # (W,NB) softmax rows
    # bins exactly as reference: floor(v / 0.1) in f32
    bins = np.minimum(NB - 1, np.floor(
        data / np.float32(0.1)).astype(np.int32))          # (B,L)

    # column-0 f64 backward pass -> per-step rescale g_t, offset C
    beta = np.ones(W, dtype=f64)
    Cacc = 0.0
    g = np.ones(L, dtype=f64)
    for t in range(L - 1, 0, -1):
        c = Ll[np.arange(W), bins[0, t]] * beta * recip[t - 1]
        tmp = we[t - 1].T @ c
        f = tmp.max()
        g[t] = 1.0 / f
        Cacc += np.log(f)
        beta = tmp * g[t]

    # row-normalization (recip) and per-step rescale (g) folded into the
    # transition weights: device matmul contracts partition k, so
    # wt'[k, t-1, i] = we[t-1, k, i] * recip[t-1, k] * g[t]
    rsg = recip.T * g[None, 1:]                            # (W, L-1)
    wtp = np.ascontiguousarray(we.transpose(1, 0, 2))      # [k,t-1,i]=we[t-1,k,i]
    wtp *= rsg[:, :, None]
    wt = wtp.astype(ml_dtypes.bfloat16)                    # (W, 255, W)

    # fp8 DoubleRow emission table: slice 0 = coarse fp8(dL), slice 1 =
    # fp8(16*(dL-coarse)) against G/16, recovering ~bf16 precision.
    dL = Ll.copy()
    dL[:, 1:] -= Ll[:, :-1]
    dLT = np.ascontiguousarray(dL.T)                       # (NB,W) f64
    coarse = dLT.astype(ml_dtypes.float8_e4m3)
    fine = ((dLT - coarse.astype(f64)) * 16.0).astype(ml_dtypes.float8_e4m3)
    dlt = np.stack([coarse, fine], axis=1)                 # (NB,2,W)

    # G[k,t,b] = [bins[b,t] >= k]   (G[0] == 1); slice 1 = G/16 (exact fp8)
    G = (bins.T[None, :, :] >= np.arange(NB)[:, None, None])
    g10 = np.empty((NB, 2, L, B), dtype=ml_dtypes.float8_e4m3)
    g10[:, 0] = G.astype(ml_dtypes.float8_e4m3)
    g10[:, 1] = (G * np.float32(0.0625)).astype(ml_dtypes.float8_e4m3)
    ones_v = np.ones((W, 1), dtype=ml_dtypes.bfloat16)
    return wt, dlt, g10, ones_v, Cacc


def kernel(data, input_distros, dense_layer_weights):
    global LAST_LNY, LAST_RESULTS, _CACHED
    wt, dlt, g10, ones_v, Cacc = _host_prep(
        np.asarray(data), np.asarray(input_distros),
        np.asarray(dense_layer_weights))

    if _CACHED is None:
        _CACHED = _build_nc()
    nc = _CACHED

    in_maps = []
    for c in range(NCORES):
        in_maps.append({
            "wt": wt, "dlt": dlt, "ones": ones_v,
            "g10": np.ascontiguousarray(g10[:, :, :, c * BC:(c + 1) * BC]),
        })
    res = run_bass_kernel_spmd(
        nc, in_maps, core_ids=list(range(NCORES)),
        trace=bool(int(os.environ.get("KERNEL_TRACE", "0"))),
        tmpdir=os.environ.get("KERNEL_TRACE_DIR") or None)
    LAST_RESULTS = res
    cs = np.concatenate([res.results[c]["colsum"].reshape(-1)
                         for c in range(NCORES)])           # (B,)
    lnY = np.log(cs.astype(np.float64)) + Cacc
    LAST_LNY = lnY
    y = np.exp(lnY).astype(np.float32).reshape(B, 1)
    return y


# revision 15
# speedup vs baseline: 1.0674x; 1.0674x over previous
"""HMM window log-likelihood on 8 NeuronCores (data-parallel over batch).

Math: reference computes, per batch column b,
    y[b] = exp(logsumexp_i x_T[b,i]),  x via log-space forward recursion.
Equivalently in linear space with row-normalized transition matrices
W_t = exp(w[t-1]) / rowsum, emission table L = softmax(distros, axis=1):
    y[b] = 1^T diag(em_T) W_T ... diag(em_1) W_1 em_0
Evaluated as a BACKWARD recursion (avoids transposing W on device):
    beta_L = 1;  beta_{t-1} = W_t'^T (em_t . beta_t)
    y[b] = sum_i em_0[i,b] beta_0[i,b]
where W_t' carries the row-normalization AND per-step rescale factors g_t
(host-computed from column 0 in f64) folded into its rows. The emission
table em[i,t,b] = L[i, bin(b,t)] is a host-side gather, streamed to SBUF
as bf16 over DMA (DMA is otherwise idle; this keeps the PE free for the
transition matmuls). Per step the device does only:
    c(t)   = em_t . beta(t)       (DVE, one PSUM operand; two 256-col chains)
    beta() = wt'[t-1]^T c(t)      (PE matmul per chain)
Device returns colsum[b] = y[b] * prod(g); host: lnY = log(colsum)+C.
The true lnY is ~ -584.6 for these inputs, so y underflows f32 to 0.0 —
exactly matching the reference (which also underflows in f32).
"""
import sys, os
for p in ("/opt/trn_rl_repo",):
    if p not in sys.path:
        sys.path.insert(0, p)
import numpy as np
import ml_dtypes

from concourse import bass, bacc, mybir
from concourse.tile import TileContext
from concourse.bass_utils import run_bass_kernel_spmd

W, L, B, NB = 128, 256, 4096, 10
NCORES = 8
BC = B // NCORES          # 512 batch cols per core
BH = BC // 2              # two chains of 256 cols
TBLK = 16                 # em streaming block (t's per DMA)

LAST_LNY = None           # debug: device-derived lnY per batch col
LAST_RESULTS = None       # debug: raw BassKernelResults

_CACHED = None            # (nc,) build cache


def _build_nc():
    nc = bacc.Bacc("TRN2", target_bir_lowering=False, debug=False,
                   num_devices=NCORES)
    bf16, f32 = mybir.dt.bfloat16, mybir.dt.float32

    wt = nc.dram_tensor("wt", [W, L - 1, W], bf16, kind="ExternalInput")
    em = nc.dram_tensor("em", [W, L, BC], bf16, kind="ExternalInput")
    ones = nc.dram_tensor("ones", [W, 1], bf16, kind="ExternalInput")
    colsum = nc.dram_tensor("colsum", [1, BC], f32, kind="ExternalOutput")

    with TileContext(nc) as tc:
        with tc.sbuf_pool(name="sb", bufs=2) as sb, \
                tc.psum_pool(name="ps", bufs=2) as ps:
            ones_sb = sb.tile([W, 1], bf16, bufs=1)
            nc.sync.dma_start(ones_sb, ones.ap())

            # all 255 transition matrices resident; chunked DMAs in backward
            # order so the scan can start as soon as the tail chunk lands
            wt_sb = sb.tile([W, L - 1, W], bf16, bufs=1)
            for cc in range((L - 1 + 7) // 8 - 1, -1, -1):
                t0 = cc * 8
                cnt = min(8, L - 1 - t0)
                nc.sync.dma_start(wt_sb[:, t0:t0 + cnt, :],
                                  wt.ap()[:, t0:t0 + cnt, :])

            em_tiles = {}

            def ensure_em(blk):
                if 0 <= blk < L // TBLK and blk not in em_tiles:
                    et = sb.tile([W, TBLK, BC], bf16, tag="em", bufs=3)
                    nc.sync.dma_start(
                        et, em.ap()[:, blk * TBLK:(blk + 1) * TBLK, :])
                    em_tiles[blk] = et

            def em_slice(t):
                blk, ti = t // TBLK, t % TBLK
                ensure_em(blk)
                ensure_em(blk - 1)      # prefetch next block (scan backward)
                return em_tiles[blk][:, ti, :]

            beta_ps = [None, None]
            cs_ps = None
            for t in range(L - 1, -1, -1):
                em_sb = em_slice(t)
                c_sb = [None, None]
                for h in (0, 1):
                    lo = h * BH
                    if t == L - 1:
                        c_sb[h] = em_sb[:, lo:lo + BH]
                    else:
                        c = sb.tile([W, BH], bf16, tag=f"c{h}", bufs=3)
                        nc.vector.tensor_mul(c, em_sb[:, lo:lo + BH],
                                             beta_ps[h])
                        c_sb[h] = c
                if t > 0:
                    for h in (0, 1):
                        b = ps.tile([W, BH], f32, tag=f"b{h}", bufs=2)
                        nc.tensor.matmul(b, wt_sb[:, t - 1, :], c_sb[h],
                                         start=True, stop=True)
                        beta_ps[h] = b
                else:
                    cs_ps = ps.tile([1, BC], f32, tag="cs", bufs=1)
                    for h in (0, 1):
                        nc.tensor.matmul(cs_ps[:, h * BH:(h + 1) * BH],
                                         ones_sb, c_sb[h],
                                         start=True, stop=True)

            cs_sb = sb.tile([1, BC], f32, bufs=1)
            nc.vector.tensor_copy(cs_sb, cs_ps)
            nc.sync.dma_start(colsum.ap(), cs_sb)
    nc.compile()
    return nc


def _host_prep(data, input_distros, dense_layer_weights):
    f64 = np.float64
    we = np.exp(dense_layer_weights.astype(f64))           # (255,W,W)
    rowsum = we.sum(axis=2)                                # (255,W)
    recip = 1.0 / rowsum
    d = input_distros.astype(f64)
    d = d - d.max(axis=1, keepdims=True)
    e = np.exp(d)
    Ll = e / e.sum(axis=1, keepdims=True)                  # (W,NB) softmax rows
    # bins exactly as reference: floor(v / 0.1) in f32
    bins = np.minimum(NB - 1, np.floor(
        data / np.float32(0.1)).astype(np.int32))          # (B,L)

    # column-0 f64 backward pass -> per-step rescale g_t, offset C
    beta = np.ones(W, dtype=f64)
    Cacc = 0.0
    g = np.ones(L, dtype=f64)
    for t in range(L - 1, 0, -1):
        c = Ll[np.arange(W), bins[0, t]] * beta * recip[t - 1]
        tmp = we[t - 1].T @ c
        f = tmp.max()
        g[t] = 1.0 / f
        Cacc += np.log(f)
        beta = tmp * g[t]

    # row-normalization (recip) and per-step rescale (g) folded into the
    # transition weights: device matmul contracts partition k, so
    # wt'[k, t-1, i] = we[t-1, k, i] * recip[t-1, k] * g[t]
    rsg = recip.T * g[None, 1:]                            # (W, L-1)
    wtp = np.ascontiguousarray(we.transpose(1, 0, 2))      # [k,t-1,i]=we[t-1,k,i]
    wtp *= rsg[:, :, None]
    wt = wtp.astype(ml_dtypes.bfloat16)                    # (W, 255, W)

    # emission table gather, per core slice: em[k, t, b] = L[k, bin(b,t)]
    Lb = Ll.astype(ml_dtypes.bfloat16)                     # (W, NB)
    ems = []
    for c in range(NCORES):
        bc = bins[c * BC:(c + 1) * BC, :].T                # (L, BC)
        ems.append(Lb[:, bc])                              # (W, L, BC)
    ones_v = np.ones((W, 1), dtype=ml_dtypes.bfloat16)
    return wt, ems, ones_v, Cacc


def kernel(data, input_distros, dense_layer_weights):
    global LAST_LNY, LAST_RESULTS, _CACHED
    wt, ems, ones_v, Cacc = _host_prep(
        np.asarray(data), np.asarray(input_distros),
        np.asarray(dense_layer_weights))

    if _CACHED is None:
        _CACHED = _build_nc()
    nc = _CACHED

    in_maps = [{"wt": wt, "em": ems[c], "ones": ones_v}
               for c in range(NCORES)]
    res = run_bass_kernel_spmd(
        nc, in_maps, core_ids=list(range(NCORES)),
        trace=bool(int(os.environ.get("KERNEL_TRACE", "0"))),
        tmpdir=os.environ.get("KERNEL_TRACE_DIR") or None)
    LAST_RESULTS = res
    cs = np.concatenate([res.results[c]["colsum"].reshape(-1)
                         for c in range(NCORES)])           # (B,)
    lnY = np.log(cs.astype(np.float64)) + Cacc
    LAST_LNY = lnY
    y = np.exp(lnY).astype(np.float32).reshape(B, 1)
    return y


# revision 16
# speedup vs baseline: 1.4509x; 1.3593x over previous
"""HMM window log-likelihood on 8 NeuronCores (data-parallel over batch).

Math: reference computes, per batch column b,
    y[b] = exp(logsumexp_i x_T[b,i]),  x via log-space forward recursion.
Equivalently in linear space with row-normalized transitions W_t and
emission table L = softmax(distros, axis=1), evaluated MEET-IN-THE-MIDDLE
so the serial dependence is ~128 steps instead of 255:
    forward   x_0 = em_0;  x_t = em_t . (Wf_t^T x_{t-1}),   t = 1..127
    backward  beta_255 = em_255;  beta_{t-1} = Wb_t^T (em_t . beta_t),
                                                            t = 255..128
    y[b] = sum_k x_127[k,b] * beta_127[k,b]
Wf/Wb carry the row-normalization and per-step rescale factors gf/gb
(host-computed from batch column 0 in f64) folded into their entries.
The emission table em[k,t,b] = L[k, bin(b,t)] is a host-side gather
streamed to SBUF as bf16 over DMA (keeps the PE free for matmuls).
Per step each direction costs one 512-wide PE matmul and one DVE
tensor-multiply (the multiply reads its beta/x operand straight from
PSUM; the two directions dovetail on the two engines).
Device returns colsum[b] = y[b] * prod(g); host: lnY = log(colsum)+C.
The true lnY is ~ -584.6 for these inputs, so y underflows f32 to 0.0 —
exactly matching the reference (which also underflows in f32).
"""
import sys, os
for p in ("/opt/trn_rl_repo",):
    if p not in sys.path:
        sys.path.insert(0, p)
import numpy as np
import ml_dtypes

from concourse import bass, bacc, mybir
from concourse.tile import TileContext
from concourse.bass_utils import run_bass_kernel_spmd

W, L, B, NB = 128, 256, 4096, 10
NCORES = 8
BC = B // NCORES          # 512 batch cols per core
M = 127                   # forward covers t=0..M, backward t=255..M+1
TBLK = 16                 # em streaming block (t's per DMA)

LAST_LNY = None           # debug: device-derived lnY per batch col
LAST_RESULTS = None       # debug: raw BassKernelResults

_CACHED = None            # (nc,) build cache


def _build_nc():
    nc = bacc.Bacc("TRN2", target_bir_lowering=False, debug=False,
                   num_devices=NCORES)
    bf16, f32 = mybir.dt.bfloat16, mybir.dt.float32

    wtb = nc.dram_tensor("wtb", [W, L - M - 1, W], bf16, kind="ExternalInput")
    wtf = nc.dram_tensor("wtf", [W, M, W], bf16, kind="ExternalInput")
    em = nc.dram_tensor("em", [W, L, BC], bf16, kind="ExternalInput")
    ones = nc.dram_tensor("ones", [W, 1], bf16, kind="ExternalInput")
    colsum = nc.dram_tensor("colsum", [1, BC], f32, kind="ExternalOutput")

    with TileContext(nc) as tc:
        with tc.sbuf_pool(name="sb", bufs=2) as sb, \
                tc.psum_pool(name="ps", bufs=2) as ps:
            ones_sb = sb.tile([W, 1], bf16, bufs=1)
            nc.sync.dma_start(ones_sb, ones.ap())

            # resident transition weights; chunked so compute starts early.
            # backward consumes wtb from index 127 down, forward wtf from 0 up
            wtb_sb = sb.tile([W, L - M - 1, W], bf16, bufs=1)
            for cc in range((L - M - 1 + 7) // 8 - 1, -1, -1):
                t0 = cc * 8
                cnt = min(8, L - M - 1 - t0)
                nc.sync.dma_start(wtb_sb[:, t0:t0 + cnt, :],
                                  wtb.ap()[:, t0:t0 + cnt, :])
            wtf_sb = sb.tile([W, M, W], bf16, bufs=1)
            for cc in range((M + 7) // 8):
                t0 = cc * 8
                cnt = min(8, M - t0)
                nc.sync.dma_start(wtf_sb[:, t0:t0 + cnt, :],
                                  wtf.ap()[:, t0:t0 + cnt, :])

            em_tiles = {}

            def ensure_em(blk):
                if 0 <= blk < L // TBLK and blk not in em_tiles:
                    tag = "emf" if blk < (M + 1) // TBLK else "emb"
                    et = sb.tile([W, TBLK, BC], bf16, tag=tag, bufs=3)
                    nc.sync.dma_start(
                        et, em.ap()[:, blk * TBLK:(blk + 1) * TBLK, :])
                    em_tiles[blk] = et

            def em_slice(t, ahead):
                blk, ti = t // TBLK, t % TBLK
                ensure_em(blk)
                ensure_em(blk + ahead)  # prefetch next block in scan order
                return em_tiles[blk][:, ti, :]

            # s = 0 boundary: backward starts from c = em_255, forward from
            # x_prev = em_0
            c_b = em_slice(L - 1, -1)
            b_ps = ps.tile([W, BC], f32, tag="bb", bufs=2)
            nc.tensor.matmul(b_ps, wtb_sb[:, L - M - 2, :], c_b,
                             start=True, stop=True)
            x_sb = em_slice(0, 1)

            for s in range(1, M + 1):
                tb = L - 1 - s          # 254..128
                tf = s                  # 1..127
                # forward matmul first so the PE works while the backward
                # multiply is still draining
                xh_ps = ps.tile([W, BC], f32, tag="xh", bufs=2)
                nc.tensor.matmul(xh_ps, wtf_sb[:, tf - 1, :], x_sb,
                                 start=True, stop=True)
                c = sb.tile([W, BC], bf16, tag="cb", bufs=3)
                nc.vector.tensor_mul(c, em_slice(tb, -1), b_ps)
                nb = ps.tile([W, BC], f32, tag="bb", bufs=2)
                nc.tensor.matmul(nb, wtb_sb[:, tb - M - 1, :], c,
                                 start=True, stop=True)
                b_ps = nb
                x = sb.tile([W, BC], bf16, tag="xf", bufs=3)
                nc.vector.tensor_mul(x, em_slice(tf, 1), xh_ps)
                x_sb = x

            # merge: y = sum_k x_127 . beta_127
            prod = sb.tile([W, BC], bf16, tag="pr", bufs=1)
            nc.vector.tensor_mul(prod, x_sb, b_ps)
            cs_ps = ps.tile([1, BC], f32, tag="cs", bufs=1)
            nc.tensor.matmul(cs_ps, ones_sb, prod, start=True, stop=True)
            cs_sb = sb.tile([1, BC], f32, bufs=1)
            nc.vector.tensor_copy(cs_sb, cs_ps)
            nc.sync.dma_start(colsum.ap(), cs_sb)
    nc.compile()
    return nc


def _host_prep(data, input_distros, dense_layer_weights):
    f64 = np.float64
    we = np.exp(dense_layer_weights.astype(f64))           # (255,W,W)
    recip = 1.0 / we.sum(axis=2)                           # (255,W)
    d = input_distros.astype(f64)
    d = d - d.max(axis=1, keepdims=True)
    e = np.exp(d)
    Ll = e / e.sum(axis=1, keepdims=True)                  # (W,NB) softmax rows
    # bins exactly as reference: floor(v / 0.1) in f32
    bins = np.minimum(NB - 1, np.floor(
        data / np.float32(0.1)).astype(np.int32))          # (B,L)

    # batch-column-0 f64 passes -> per-step rescales gf/gb, offset C
    x = Ll[:, bins[0, 0]].copy()
    Cf = 0.0
    gf = np.ones(L, f64)
    for t in range(1, M + 1):
        xh = (we[t - 1] * recip[t - 1][:, None]) @ x
        xh = xh * Ll[:, bins[0, t]]
        f = xh.max()
        gf[t] = 1.0 / f
        Cf += np.log(f)
        x = xh * gf[t]
    beta = np.ones(W, dtype=f64)
    Cb = 0.0
    gb = np.ones(L, f64)
    for t in range(L - 1, M, -1):
        c = Ll[:, bins[0, t]] * beta * recip[t - 1]
        tmp = we[t - 1].T @ c
        f = tmp.max()
        gb[t] = 1.0 / f
        Cb += np.log(f)
        beta = tmp * gb[t]

    # fold normalization + rescales into the transition weights.
    # backward mm at t (128..255): contracts partition k (rows of we[t-1]):
    #   wtb[k, t-128, i] = we[t-1, k, i] * recip[t-1, k] * gb[t]
    Ab = we[M:] * (recip[M:, :, None] * gb[M + 1:, None, None])
    wtb = np.ascontiguousarray(Ab.transpose(1, 0, 2)).astype(ml_dtypes.bfloat16)
    # forward mm at t (1..127): x_t[i] = em*sum_j W[i,j]x[j]:
    #   wtf[j, t-1, i] = we[t-1, i, j] * recip[t-1, i] * gf[t]
    Af = we[:M] * (recip[:M, :, None] * gf[1:M + 1, None, None])
    wtf = np.ascontiguousarray(Af.transpose(2, 0, 1)).astype(ml_dtypes.bfloat16)

    # emission table gather, per core slice: em[k, t, b] = L[k, bin(b,t)]
    Lb = Ll.astype(ml_dtypes.bfloat16)                     # (W, NB)
    ems = []
    for c in range(NCORES):
        bc = bins[c * BC:(c + 1) * BC, :].T                # (L, BC)
        ems.append(Lb[:, bc])                              # (W, L, BC)
    ones_v = np.ones((W, 1), dtype=ml_dtypes.bfloat16)
    return wtb, wtf, ems, ones_v, Cf + Cb


def kernel(data, input_distros, dense_layer_weights):
    global LAST_LNY, LAST_RESULTS, _CACHED
    wtb, wtf, ems, ones_v, Cacc = _host_prep(
        np.asarray(data), np.asarray(input_distros),
        np.asarray(dense_layer_weights))

    if _CACHED is None:
        _CACHED = _build_nc()
    nc = _CACHED

    in_maps = [{"wtb": wtb, "wtf": wtf, "em": ems[c], "ones": ones_v}
               for c in range(NCORES)]
    res = run_bass_kernel_spmd(
        nc, in_maps, core_ids=list(range(NCORES)),
        trace=bool(int(os.environ.get("KERNEL_TRACE", "0"))),
        tmpdir=os.environ.get("KERNEL_TRACE_DIR") or None)
    LAST_RESULTS = res
    cs = np.concatenate([res.results[c]["colsum"].reshape(-1)
                         for c in range(NCORES)])           # (B,)
    lnY = np.log(cs.astype(np.float64)) + Cacc
    LAST_LNY = lnY
    y = np.exp(lnY).astype(np.float32).reshape(B, 1)
    return y


# revision 18
# speedup vs baseline: 1.5734x; 1.0844x over previous
"""HMM window log-likelihood on 8 NeuronCores (data-parallel over batch).

Math: reference computes, per batch column b,
    y[b] = exp(logsumexp_i x_T[b,i]),  x via log-space forward recursion.
Equivalently in linear space with row-normalized transitions W_t and
emission table L = softmax(distros, axis=1), evaluated MEET-IN-THE-MIDDLE
so the serial dependence is ~128 steps instead of 255:
    forward   x_0 = em_0;  x_t = em_t . (Wf_t^T x_{t-1}),   t = 1..127
    backward  beta_255 = em_255;  beta_{t-1} = Wb_t^T (em_t . beta_t),
                                                            t = 255..128
    y[b] = sum_k x_127[k,b] * beta_127[k,b]
Wf/Wb carry the row-normalization and per-step rescale factors gf/gb
(host-computed from batch column 0 in f64) folded into their entries.
The emission table em[k,t,b] = L[k, bin(b,t)] is a host-side gather
streamed to SBUF as bf16 over DMA (keeps the PE free for matmuls).
Per step each direction costs one 512-wide PE matmul and one DVE
tensor-multiply (the multiply reads its beta/x operand straight from
PSUM; the two directions dovetail on the two engines).
Device returns colsum[b] = y[b] * prod(g); host: lnY = log(colsum)+C.
The true lnY is ~ -584.6 for these inputs, so y underflows f32 to 0.0 —
exactly matching the reference (which also underflows in f32).
"""
import sys, os
for p in ("/opt/trn_rl_repo",):
    if p not in sys.path:
        sys.path.insert(0, p)
import numpy as np
import ml_dtypes

from concourse import bass, bacc, mybir
from concourse.tile import TileContext
from concourse.bass_utils import run_bass_kernel_spmd

W, L, B, NB = 128, 256, 4096, 10
NCORES = 8
BC = B // NCORES          # 512 batch cols per core
M = 127                   # forward covers t=0..M, backward t=255..M+1
TBLK = 8                  # em streaming block (t's per DMA)

LAST_LNY = None           # debug: device-derived lnY per batch col
LAST_RESULTS = None       # debug: raw BassKernelResults

_CACHED = None            # (nc,) build cache


def _build_nc():
    nc = bacc.Bacc("TRN2", target_bir_lowering=False, debug=False,
                   num_devices=NCORES)
    bf16, f32 = mybir.dt.bfloat16, mybir.dt.float32

    wtb = nc.dram_tensor("wtb", [W, L - M - 1, W], bf16, kind="ExternalInput")
    wtf = nc.dram_tensor("wtf", [W, M, W], bf16, kind="ExternalInput")
    em = nc.dram_tensor("em", [W, L, BC], bf16, kind="ExternalInput")
    ones = nc.dram_tensor("ones", [W, 1], bf16, kind="ExternalInput")
    colsum = nc.dram_tensor("colsum", [1, BC], f32, kind="ExternalOutput")

    with TileContext(nc) as tc:
        with tc.sbuf_pool(name="sb", bufs=2) as sb, \
                tc.psum_pool(name="ps", bufs=2) as ps:
            ones_sb = sb.tile([W, 1], bf16, bufs=1)
            nc.sync.dma_start(ones_sb, ones.ap())

            em_tiles = {}

            def ensure_em(blk):
                if 0 <= blk < L // TBLK and blk not in em_tiles:
                    tag = "emf" if blk < (M + 1) // TBLK else "emb"
                    et = sb.tile([W, TBLK, BC], bf16, tag=tag, bufs=3)
                    nc.sync.dma_start(
                        et, em.ap()[:, blk * TBLK:(blk + 1) * TBLK, :])
                    em_tiles[blk] = et

            def em_slice(t, ahead):
                blk, ti = t // TBLK, t % TBLK
                ensure_em(blk)
                ensure_em(blk + ahead)  # prefetch next block in scan order
                return em_tiles[blk][:, ti, :]

            # the first em block of each direction gates the first compute —
            # issue those DMAs before the bulk weight loads
            ensure_em(L // TBLK - 1)
            ensure_em(0)

            # resident transition weights; chunks interleaved in consumption
            # order (backward uses wtb from index 127 down, forward wtf from
            # 0 up) so both chains can start as soon as their head chunk lands
            wtb_sb = sb.tile([W, L - M - 1, W], bf16, bufs=1)
            wtf_sb = sb.tile([W, M, W], bf16, bufs=1)
            nb_ch = (L - M - 1 + 7) // 8
            nf_ch = (M + 7) // 8
            for k in range(max(nb_ch, nf_ch)):
                if k < nb_ch:
                    t0 = (nb_ch - 1 - k) * 8
                    cnt = min(8, L - M - 1 - t0)
                    nc.sync.dma_start(wtb_sb[:, t0:t0 + cnt, :],
                                      wtb.ap()[:, t0:t0 + cnt, :])
                if k < nf_ch:
                    t0 = k * 8
                    cnt = min(8, M - t0)
                    nc.sync.dma_start(wtf_sb[:, t0:t0 + cnt, :],
                                      wtf.ap()[:, t0:t0 + cnt, :])

            # s = 0 boundary: backward starts from c = em_255, forward from
            # x_prev = em_0
            c_b = em_slice(L - 1, -1)
            b_ps = ps.tile([W, BC], f32, tag="bb", bufs=2)
            nc.tensor.matmul(b_ps, wtb_sb[:, L - M - 2, :], c_b,
                             start=True, stop=True)
            x_sb = em_slice(0, 1)

            for s in range(1, M + 1):
                tb = L - 1 - s          # 254..128
                tf = s                  # 1..127
                # forward matmul first so the PE works while the backward
                # multiply is still draining
                xh_ps = ps.tile([W, BC], f32, tag="xh", bufs=2)
                nc.tensor.matmul(xh_ps, wtf_sb[:, tf - 1, :], x_sb,
                                 start=True, stop=True)
                c = sb.tile([W, BC], bf16, tag="cb", bufs=3)
                nc.vector.tensor_mul(c, em_slice(tb, -1), b_ps)
                nb = ps.tile([W, BC], f32, tag="bb", bufs=2)
                nc.tensor.matmul(nb, wtb_sb[:, tb - M - 1, :], c,
                                 start=True, stop=True)
                b_ps = nb
                x = sb.tile([W, BC], bf16, tag="xf", bufs=3)
                nc.vector.tensor_mul(x, em_slice(tf, 1), xh_ps)
                x_sb = x

            # merge: y = sum_k x_127 . beta_127
            prod = sb.tile([W, BC], bf16, tag="pr", bufs=1)
            nc.vector.tensor_mul(prod, x_sb, b_ps)
            cs_ps = ps.tile([1, BC], f32, tag="cs", bufs=1)
            nc.tensor.matmul(cs_ps, ones_sb, prod, start=True, stop=True)
            cs_sb = sb.tile([1, BC], f32, bufs=1)
            nc.vector.tensor_copy(cs_sb, cs_ps)
            nc.sync.dma_start(colsum.ap(), cs_sb)
    nc.compile()
    return nc


def _host_prep(data, input_distros, dense_layer_weights):
    f64 = np.float64
    we = np.exp(dense_layer_weights.astype(f64))           # (255,W,W)
    recip = 1.0 / we.sum(axis=2)                           # (255,W)
    d = input_distros.astype(f64)
    d = d - d.max(axis=1, keepdims=True)
    e = np.exp(d)
    Ll = e / e.sum(axis=1, keepdims=True)                  # (W,NB) softmax rows
    # bins exactly as reference: floor(v / 0.1) in f32
    bins = np.minimum(NB - 1, np.floor(
        data / np.float32(0.1)).astype(np.int32))          # (B,L)

    # batch-column-0 f64 passes -> per-step rescales gf/gb, offset C
    x = Ll[:, bins[0, 0]].copy()
    Cf = 0.0
    gf = np.ones(L, f64)
    for t in range(1, M + 1):
        xh = (we[t - 1] * recip[t - 1][:, None]) @ x
        xh = xh * Ll[:, bins[0, t]]
        f = xh.max()
        gf[t] = 1.0 / f
        Cf += np.log(f)
        x = xh * gf[t]
    beta = np.ones(W, dtype=f64)
    Cb = 0.0
    gb = np.ones(L, f64)
    for t in range(L - 1, M, -1):
        c = Ll[:, bins[0, t]] * beta * recip[t - 1]
        tmp = we[t - 1].T @ c
        f = tmp.max()
        gb[t] = 1.0 / f
        Cb += np.log(f)
        beta = tmp * gb[t]

    # fold normalization + rescales into the transition weights.
    # backward mm at t (128..255): contracts partition k (rows of we[t-1]):
    #   wtb[k, t-128, i] = we[t-1, k, i] * recip[t-1, k] * gb[t]
    Ab = we[M:] * (recip[M:, :, None] * gb[M + 1:, None, None])
    wtb = np.ascontiguousarray(Ab.transpose(1, 0, 2)).astype(ml_dtypes.bfloat16)
    # forward mm at t (1..127): x_t[i] = em*sum_j W[i,j]x[j]:
    #   wtf[j, t-1, i] = we[t-1, i, j] * recip[t-1, i] * gf[t]
    Af = we[:M] * (recip[:M, :, None] * gf[1:M + 1, None, None])
    wtf = np.ascontiguousarray(Af.transpose(2, 0, 1)).astype(ml_dtypes.bfloat16)

    # emission table gather, per core slice: em[k, t, b] = L[k, bin(b,t)]
    Lb = Ll.astype(ml_dtypes.bfloat16)                     # (W, NB)
    ems = []
    for c in range(NCORES):
        bc = bins[c * BC:(c + 1) * BC, :].T                # (L, BC)
        ems.append(Lb[:, bc])                              # (W, L, BC)
    ones_v = np.ones((W, 1), dtype=ml_dtypes.bfloat16)
    return wtb, wtf, ems, ones_v, Cf + Cb


def kernel(data, input_distros, dense_layer_weights):
    global LAST_LNY, LAST_RESULTS, _CACHED
    wtb, wtf, ems, ones_v, Cacc = _host_prep(
        np.asarray(data), np.asarray(input_distros),
        np.asarray(dense_layer_weights))

    if _CACHED is None:
        _CACHED = _build_nc()
    nc = _CACHED

    in_maps = [{"wtb": wtb, "wtf": wtf, "em": ems[c], "ones": ones_v}
               for c in range(NCORES)]
    res = run_bass_kernel_spmd(
        nc, in_maps, core_ids=list(range(NCORES)),
        trace=bool(int(os.environ.get("KERNEL_TRACE", "0"))),
        tmpdir=os.environ.get("KERNEL_TRACE_DIR") or None)
    LAST_RESULTS = res
    cs = np.concatenate([res.results[c]["colsum"].reshape(-1)
                         for c in range(NCORES)])           # (B,)
    lnY = np.log(cs.astype(np.float64)) + Cacc
    LAST_LNY = lnY
    y = np.exp(lnY).astype(np.float32).reshape(B, 1)
    return y
